# revision 1
# baseline (speedup 1.0000x reference)
"""ContinuousMask kernel for Trainium2 (8 NeuronCores, SPMD row-sharded).

Problem: starts[B=2048, N=8192] int32, T=16384, l=1638. Output bool [B, T]:
True everywhere except the union of windows [s, s+l) over each row's starts.

Algorithm (per row):
  A position t is covered iff some start lies in (t-l, t]. With value-chunks
  of width W=512 (2W <= l), if every chunk 0..(smax>>9)-1 contains at least
  one start, then the covered region is EXACTLY [smin, smax+l):
    - t in [smin, smin+l): covered by the smin window.
    - t in [smin+l, smax): the previous chunk of t is nonempty; any start s'
      there satisfies t-l < s' <= t (since 2W <= l).
    - t in [smax, smax+l): covered by the smax window.
    - t < smin or t >= smax+l: no start in (t-l, t].
  The device computes smin, smax (full reduces) and an exact chunk-occupancy
  bitmask over a WITNESS SUBSET of columns (subset occupancy passing PROVES
  the condition; failing only flags the row for exact host recompute — on the
  target distribution a 2048-column witness fails with P ~ 1e-26). The
  occupancy requirement is strengthened to chunks 0..25 so that a passing row
  also has smin < 512 and smax >= 12800, which bounds the True runs to the
  painted head/tail strips. Flagged rows are recomputed exactly on host.

  The constant-zero middle of the mask is never stored: run_bass_kernel_spmd
  (both native and PJRT/axon paths) guarantees ExternalOutput buffers are
  zero-initialized (pre-zeroed / donated zero buffers), so only the head and
  tail strips are written.
"""

import numpy as np

B = 2048
T = 16384
NSEG = 8192
L = 1638
NCORES = 8
RPC = B // NCORES  # 256 rows per core
PT = 128  # rows per partition tile
NRT = RPC // PT  # 2 row tiles per core
SHIFT = 9  # occupancy chunk width 512 (2*512 <= L)
OCC_COLS = 1024  # occupancy witness column count (chunk-28 expectation ~28 hits)
# Require witness occupancy of ALL chunks 0..28 (values span [0, 14747), so
# chunk 28 is the last). Chunk 0 occupied => smin < 512; chunk 28 occupied =>
# smax >= 14336 => the tail True-run starts at smax+L >= 15974. Chunk 28's
# witness expectation is ~57 hits (P(flag) ~ e^-57 per row).
MIN_CLAST = 29
HSTRIP = 512  # head strip [0, 512) covers [0, smin) since smin < 512
TSTART = T - 512  # tail strip [15872, T) covers runs starting >= 15974

_prog_cache: dict = {}


def _build_program(reps: int = 1, mode: str = "full"):
    """mode: 'full' | 'dma' (loads+stores only) | 'compute' (load once, compute reps x)."""
    import concourse.bacc as bacc
    import concourse.mybir as mybir
    from concourse.tile import TileContext

    dt = mybir.dt
    Alu = mybir.AluOpType
    X = mybir.AxisListType.X

    nc = bacc.Bacc("TRN2", debug=False)
    starts_d = nc.declare_dram_parameter("starts", [RPC, NSEG], dt.int32, isOutput=False)
    mask_d = nc.declare_dram_parameter("mask", [RPC, T], dt.uint8, isOutput=True)
    flags_d = nc.declare_dram_parameter("flags", [RPC, 1], dt.int32, isOutput=True)

    HALF = NSEG // 2
    with TileContext(nc) as tc:
        with (
            tc.tile_pool(name="persist", bufs=1) as pp,
            tc.tile_pool(name="stp", bufs=2) as stp,
            tc.tile_pool(name="strip", bufs=4) as outp,
            tc.tile_pool(name="work", bufs=1) as wp,
            tc.tile_pool(name="small", bufs=4) as sp,
        ):
            iota_t = pp.tile([PT, HSTRIP], dt.int16, tag="iota")
            nc.gpsimd.iota(iota_t[:], [[1, HSTRIP]], base=0, channel_multiplier=0)
            ones_t = pp.tile([PT, OCC_COLS], dt.int32, tag="ones")
            nc.vector.memset(ones_t[:], 1)

            persist_st: dict = {}
            for rep in range(reps):
              for rt in range(NRT):
                r0 = rt * PT
                do_load = mode != "compute" or rep == 0
                do_compute = mode != "dma"
                do_store = mode != "compute"

                if mode == "compute":
                    if rt not in persist_st:
                        st_persist = pp.tile([PT, NSEG], dt.int32, tag=f"st{rt}")
                        persist_st[rt] = st_persist
                    st = persist_st[rt]
                else:
                    st = stp.tile([PT, NSEG], dt.int32, tag="st")
                if do_load:
                    # two half-loads so reduces can start at half-load
                    nc.sync.dma_start(out=st[:, 0:HALF], in_=starts_d[r0 : r0 + PT, 0:HALF])
                    nc.sync.dma_start(out=st[:, HALF:NSEG], in_=starts_d[r0 : r0 + PT, HALF:NSEG])
                if not do_compute:
                    if do_store:
                        ph0 = outp.tile([PT, HSTRIP], dt.uint8, tag="ph")
                        nc.vector.memset(ph0[:], 0)
                        nc.scalar.dma_start(out=mask_d[r0 : r0 + PT, 0:HSTRIP], in_=ph0[:])
                        pt0 = outp.tile([PT, T - TSTART], dt.uint8, tag="pt")
                        nc.vector.memset(pt0[:], 0)
                        nc.scalar.dma_start(out=mask_d[r0 : r0 + PT, TSTART:T], in_=pt0[:])
                    continue

                # exact per-row min/max: partial reduce per half-load, combine
                smin = sp.tile([PT, 1], dt.int32, tag="smin")
                smax = sp.tile([PT, 1], dt.int32, tag="smax")
                mn1 = sp.tile([PT, 1], dt.int32, tag="mn1")
                mx1 = sp.tile([PT, 1], dt.int32, tag="mx1")
                nc.vector.tensor_reduce(smin[:], st[:, 0:HALF], X, Alu.min)
                nc.vector.tensor_reduce(smax[:], st[:, 0:HALF], X, Alu.max)
                nc.vector.tensor_reduce(mn1[:], st[:, HALF:NSEG], X, Alu.min)
                nc.vector.tensor_reduce(mx1[:], st[:, HALF:NSEG], X, Alu.max)
                nc.vector.tensor_tensor(smin[:], smin[:], mn1[:], Alu.min)
                nc.vector.tensor_tensor(smax[:], smax[:], mx1[:], Alu.max)

                # witness occupancy bitmask over the first OCC_COLS columns
                hi = wp.tile([PT, OCC_COLS], dt.int32, tag="hi")
                nc.vector.tensor_scalar(hi[:], st[:, 0:OCC_COLS], SHIFT, None, Alu.arith_shift_right)
                bits = wp.tile([PT, OCC_COLS], dt.int32, tag="bits")
                nc.vector.tensor_tensor(bits[:], ones_t[:], hi[:], Alu.logical_shift_left)
                w = OCC_COLS
                while w > 1:
                    h = w // 2
                    nc.vector.tensor_tensor(
                        bits[:, 0:h], bits[:, 0:h], bits[:, h:w], Alu.bitwise_or
                    )
                    w = h

                # flag = (occ | (-1 << MIN_CLAST)) != -1. Since MIN_CLAST=29
                # exceeds any clast (smax>>9 <= 28), max(clast, MIN_CLAST) is the
                # constant 29, so the mask is compile-time: -1<<29 = -2^29 (fp32-
                # exact immediate). Pure bitwise + fp32-safe compare.
                bad = sp.tile([PT, 1], dt.int32, tag="bad")
                nc.vector.tensor_scalar(bad[:], bits[:, 0:1], -(1 << MIN_CLAST), None, Alu.bitwise_or)
                nc.vector.tensor_scalar(bad[:], bad[:], -1.0, None, Alu.not_equal)
                if do_store:
                    nc.scalar.dma_start(out=flags_d[r0 : r0 + PT, :], in_=bad[:])

                # paint strips: head (t < smin) on DVE, tail (t >= smax+L-TSTART)
                # on GPSIMD; scalars prepared on ScalarE
                smin_f = sp.tile([PT, 1], dt.float32, tag="sminf")
                nc.scalar.copy(smin_f[:], smin[:])
                smaxl_f = sp.tile([PT, 1], dt.float32, tag="smaxlf")
                nc.scalar.activation(
                    smaxl_f[:], smax[:], mybir.ActivationFunctionType.Copy,
                    bias=float(L - TSTART), scale=1.0,
                )
                ph = outp.tile([PT, HSTRIP], dt.uint8, tag="ph")
                pt = outp.tile([PT, T - TSTART], dt.uint8, tag="pt")
                nc.vector.tensor_scalar(ph[:], iota_t[:], smin_f[:], None, Alu.is_lt)
                nc.gpsimd.tensor_scalar(pt[:], iota_t[:], smaxl_f[:], None, Alu.is_ge)
                if do_store:
                    nc.scalar.dma_start(out=mask_d[r0 : r0 + PT, 0:HSTRIP], in_=ph[:])
                    nc.scalar.dma_start(out=mask_d[r0 : r0 + PT, TSTART:T], in_=pt[:])

    nc.finalize()
    return nc


def _get_program(reps: int = 1, mode: str = "full"):
    key = (reps, mode)
    if key not in _prog_cache:
        _prog_cache[key] = _build_program(reps, mode)
    return _prog_cache[key]


def _host_exact_row(row_starts: np.ndarray) -> np.ndarray:
    delta = np.zeros(T + 1, np.int64)
    np.add.at(delta, row_starts, 1)
    np.add.at(delta, row_starts + L, -1)
    return ~(np.cumsum(delta)[:T] > 0)


def run_device(starts: np.ndarray, trace: bool = False):
    """Run the SPMD bass kernel. Returns (mask_u8 [B,T], flags [B], results)."""
    from concourse.bass_utils import run_bass_kernel_spmd

    nc = _get_program()
    shards = starts.reshape(NCORES, RPC, NSEG)
    in_maps = [{"starts": np.ascontiguousarray(shards[c])} for c in range(NCORES)]
    res = run_bass_kernel_spmd(nc, in_maps, list(range(NCORES)), trace=trace)
    mask = np.concatenate([r["mask"] for r in res.results], axis=0)
    flags = np.concatenate([r["flags"] for r in res.results], axis=0).reshape(-1)
    return mask, flags, res


def kernel(**inputs) -> np.ndarray:
    starts = np.ascontiguousarray(np.asarray(inputs["starts"]), dtype=np.int32)
    t_in = int(np.asarray(inputs["T"]))
    l_in = int(np.asarray(inputs["l"]))
    assert starts.shape == (B, NSEG), starts.shape
    assert t_in == T and l_in == L, (t_in, l_in)

    mask_u8, flags, _ = run_device(starts)
    mask = mask_u8.astype(bool)

    bad_rows = np.nonzero(flags != 0)[0]
    for r in bad_rows:  # pathological rows: exact host recompute (never on real data)
        mask[r] = _host_exact_row(starts[r])
    return mask



# revision 17
# speedup vs baseline: 1.6516x; 1.6516x over previous
"""ContinuousMask kernel for Trainium2 (8 NeuronCores, SPMD row-sharded).

Problem: starts[B=2048, N=8192] int32, T=16384, l=1638. Output bool [B, T]:
True everywhere except the union of windows [s, s+l) over each row's starts.

Algorithm (per row):
  A position t is covered iff some start lies in (t-l, t]. With value-chunks
  of width W=512 (2W <= l), if every chunk 0..28 contains at least one start,
  then the covered region is EXACTLY [smin, smax+l), smin < 512, and
  smax+l >= 15974, so the mask is fully described by a head strip [0, 512)
  (True iff t < smin) and a tail strip [TSTART, T) (True iff t >= smax+l);
  the constant-False middle is never stored (run_bass_kernel_spmd's PJRT
  path donates zero-initialized output buffers). Chunk occupancy is checked
  on a WITNESS SUBSET of columns (passing PROVES the condition; failing only
  flags the row for exact host recompute — on the target distribution a
  512-column witness flags a row with P ~ 5e-7).

Engine split (per 128-row tile), tuned against the TRN2 cost model:
  - loads: 4 quarter-loads, interleaved across both HWDGE queues (SP + Act).
  - DVE (the critical engine, ~1 int32 elem/cycle/lane): min and max via
    fused tensor_tensor_reduce over quarter PAIRS (2 elems/cycle), chained
    through the accumulator-init operand so compute starts when the first
    half lands; witness shift; fused shift-left + or-reduce for occupancy;
    flag test; head-strip paint.
  - Pool (gpsimd): tail-strip paint (integer max/shift are DVE-only).
  - Act (scalar): per-row scalar prep + store DMAs + half the loads.
"""

import numpy as np

B = 2048
T = 16384
NSEG = 8192
L = 1638
NCORES = 8
RPC = B // NCORES  # 256 rows per core
PT = 128  # rows per partition tile
NRT = RPC // PT  # 2 row tiles per core
Q = NSEG // 4  # quarter width (2048)
SHIFT = 9  # occupancy chunk width 512 (2*512 <= L)
OCC_COLS = 512  # occupancy witness column count
# Require witness occupancy of ALL chunks 0..28 (values span [0, 14747), so
# chunk 28 is the last). Chunk 0 occupied => smin < 512; chunk 28 occupied =>
# smax >= 14336 => the tail True-run starts at smax+L >= 15974.
MIN_CLAST = 29
HSTRIP = 512  # head strip [0, 512) covers [0, smin) since smin < 512
TSTART = T - 512  # tail strip [15872, T) covers runs starting >= 15974

_prog_cache: dict = {}


def _register_minmax_ops():
    """Register two-stream min/max reduce custom DVE ops (per-NEFF uop table;
    the documented extension path — no firmware change). The stock
    InstTensorTensorReduce wedges this hardware, so these replace it:
    one instruction streams two operand ranges (2 elems/cycle/lane) and
    folds the pairwise result into the fp32 accumulator."""
    import concourse.dve_ops as dve_ops
    from concourse.dve_ops import DveOp
    from concourse.dve_spec import Spec, Src0, Src1, C0, minn, maxx, lower
    from concourse.dve_spec import _has_src1
    from concourse.dve_uop import DveOpSpec

    existing = {op.name: op for op in dve_ops.OPS}
    if "TT_MIN_REDUCE_X" in existing:
        return existing["TT_MIN_REDUCE_X"], existing["TT_MAX_REDUCE_X"]

    def _ref(np_op):
        fold = np.min if np_op is np.minimum else np.max

        def ref(in0, in1, c0, c1, c2):
            out = np_op(
                np.asarray(in0).astype(np.float32), np.asarray(in1).astype(np.float32)
            )
            acc = np_op(np.asarray(c0, np.float32), fold(out, axis=-1, keepdims=True))
            return out, acc

        return ref

    def make(name, body, accum, np_op):
        spec = Spec(body=body, accum=accum, accum_init=C0, reference=_ref(np_op))
        row = 1 + len(dve_ops.OPS)
        assert row < 0x20, "custom DVE row overflow"
        dve_ops._SUB_OPCODE_FOR_NAME[name] = row
        uops = lower(spec, ver="v3")
        sha = DveOpSpec(
            name=name, opcode=row, uops=uops, rd1_en=_has_src1(spec)
        ).sha("v3")
        op = DveOp(name=name, spec=spec, subdim=False, uops_sha={"v3": sha})
        dve_ops.OPS.append(op)
        dve_ops.CUSTOM_DVE_SPECS[name] = spec
        return op

    mn = make("TT_MIN_REDUCE_X", minn(Src0, Src1), minn, np.minimum)
    mx = make("TT_MAX_REDUCE_X", maxx(Src0, Src1), maxx, np.maximum)
    return mn, mx


def _build_program(reps: int = 1, mode: str = "full"):
    """mode: 'full' | 'dma' (loads+stores only) | 'compute' (load once, compute reps x)."""
    import concourse.bacc as bacc
    import concourse.mybir as mybir
    from concourse.tile import TileContext

    MN_OP, MX_OP = _register_minmax_ops()

    dt = mybir.dt
    Alu = mybir.AluOpType
    X = mybir.AxisListType.X

    nc = bacc.Bacc("TRN2", debug=False)
    starts_d = nc.declare_dram_parameter("starts", [RPC, NSEG], dt.int32, isOutput=False)
    mask_d = nc.declare_dram_parameter("mask", [RPC, T], dt.uint8, isOutput=True)
    flags_d = nc.declare_dram_parameter("flags", [RPC, 32], dt.int32, isOutput=True)

    with TileContext(nc) as tc:
        with (
            tc.tile_pool(name="persist", bufs=1) as pp,
            tc.tile_pool(name="stp", bufs=2) as stp,
            tc.tile_pool(name="scratch", bufs=2) as scp,
            tc.tile_pool(name="strip", bufs=4) as outp,
            tc.tile_pool(name="small", bufs=4) as sp,
        ):
            iota_t = pp.tile([PT, HSTRIP], dt.int16, tag="iota")
            nc.gpsimd.iota(iota_t[:], [[1, HSTRIP]], base=0, channel_multiplier=0)
            ones_t = pp.tile([PT, OCC_COLS], dt.int32, tag="ones")
            nc.vector.memset(ones_t[:], 1)

            persist_st: dict = {}
            for rep in range(reps):
              for rt in range(NRT):
                r0 = rt * PT
                do_load = mode != "compute" or rep == 0
                do_compute = mode != "dma"
                do_store = mode != "compute"

                if mode == "compute":
                    if rt not in persist_st:
                        st_persist = pp.tile([PT, NSEG], dt.int32, tag=f"st{rt}")
                        persist_st[rt] = st_persist
                    st = persist_st[rt]
                else:
                    st = stp.tile([PT, NSEG], dt.int32, tag="st")
                if do_load:
                    # quarter-loads, interleaved across both HWDGE queues so
                    # the first ttr pair can start at the half-way point
                    nc.sync.dma_start(out=st[:, 0:Q], in_=starts_d[r0 : r0 + PT, 0:Q])
                    nc.scalar.dma_start(out=st[:, Q : 2 * Q], in_=starts_d[r0 : r0 + PT, Q : 2 * Q])
                    nc.sync.dma_start(out=st[:, 2 * Q : 3 * Q], in_=starts_d[r0 : r0 + PT, 2 * Q : 3 * Q])
                    nc.scalar.dma_start(out=st[:, 3 * Q : NSEG], in_=starts_d[r0 : r0 + PT, 3 * Q : NSEG])
                if not do_compute:
                    if do_store:
                        ph0 = outp.tile([PT, HSTRIP], dt.uint8, tag="ph")
                        nc.vector.memset(ph0[:], 0)
                        nc.scalar.dma_start(out=mask_d[r0 : r0 + PT, 0:HSTRIP], in_=ph0[:])
                        pt0 = outp.tile([PT, T - TSTART], dt.uint8, tag="pt")
                        nc.vector.memset(pt0[:], 0)
                        nc.scalar.dma_start(out=mask_d[r0 : r0 + PT, TSTART:T], in_=pt0[:])
                    continue

                # fp32 accumulators: the DVE reduce accumulator is fp32; all
                # values here are < 2^20 so fp32 is exact
                smin = sp.tile([PT, 1], dt.float32, tag="smin")
                smax = sp.tile([PT, 1], dt.float32, tag="smax")
                mn0 = sp.tile([PT, 1], dt.float32, tag="mn0")
                mx0 = sp.tile([PT, 1], dt.float32, tag="mx0")
                dmy = sp.tile([PT, 1], dt.float32, tag="dmy")

                # witness occupancy shift (needs only q0)
                hi = scp.tile([PT, OCC_COLS], dt.int32, tag="hi")
                nc.vector.tensor_scalar(hi[:], st[:, 0:OCC_COLS], SHIFT, None, Alu.arith_shift_right)

                # min/max over quarter pairs (q0,q1) then (q2,q3), chaining
                # through the accumulator initial value; the elementwise
                # output is discarded via a stride-0 broadcast dummy
                nc.vector._custom_dve(
                    MN_OP, out=dmy.broadcast_to((PT, Q)),
                    in0=st[:, 0:Q], in1=st[:, Q : 2 * Q],
                    s0=float(1 << 20), accum_out=mn0[:],
                )
                nc.vector._custom_dve(
                    MX_OP, out=dmy.broadcast_to((PT, Q)),
                    in0=st[:, 0:Q], in1=st[:, Q : 2 * Q],
                    s0=0.0, accum_out=mx0[:],
                )
                nc.vector._custom_dve(
                    MN_OP, out=dmy.broadcast_to((PT, Q)),
                    in0=st[:, 2 * Q : 3 * Q], in1=st[:, 3 * Q : NSEG],
                    s0=mn0[:], accum_out=smin[:],
                )
                nc.vector._custom_dve(
                    MX_OP, out=dmy.broadcast_to((PT, Q)),
                    in0=st[:, 2 * Q : 3 * Q], in1=st[:, 3 * Q : NSEG],
                    s0=mx0[:], accum_out=smax[:],
                )

                bits = scp.tile([PT, OCC_COLS], dt.int32, tag="bits")
                nc.vector.tensor_tensor(bits[:], ones_t[:], hi[:], Alu.logical_shift_left)
                # or-tree down to 32 columns; the final OR + flag test is done
                # on host (tensor_reduce has no bitwise_or)
                w = OCC_COLS
                while w > 32:
                    h = w // 2
                    nc.vector.tensor_tensor(
                        bits[:, 0:h], bits[:, 0:h], bits[:, h:w], Alu.bitwise_or
                    )
                    w = h
                if do_store:
                    nc.scalar.dma_start(out=flags_d[r0 : r0 + PT, :], in_=bits[:, 0:32])

                # paint strips: head (t < smin) on DVE, tail (t >= smax+L-TSTART)
                # on GPSIMD; smin/smax are already fp32 so only the tail needs
                # a bias-add, done on ScalarE
                smaxl_f = sp.tile([PT, 1], dt.float32, tag="smaxlf")
                nc.scalar.activation(
                    smaxl_f[:], smax[:], mybir.ActivationFunctionType.Copy,
                    bias=float(L - TSTART), scale=1.0,
                )
                ph = outp.tile([PT, HSTRIP], dt.uint8, tag="ph")
                pt = outp.tile([PT, T - TSTART], dt.uint8, tag="pt")
                nc.vector.tensor_scalar(ph[:], iota_t[:], smin[:], None, Alu.is_lt)
                nc.gpsimd.tensor_scalar(pt[:], iota_t[:], smaxl_f[:], None, Alu.is_ge)
                if do_store:
                    nc.scalar.dma_start(out=mask_d[r0 : r0 + PT, 0:HSTRIP], in_=ph[:])
                    nc.scalar.dma_start(out=mask_d[r0 : r0 + PT, TSTART:T], in_=pt[:])

    nc.finalize()
    return nc


def _get_program(reps: int = 1, mode: str = "full"):
    key = (reps, mode)
    if key not in _prog_cache:
        _prog_cache[key] = _build_program(reps, mode)
    return _prog_cache[key]


def _host_exact_row(row_starts: np.ndarray) -> np.ndarray:
    delta = np.zeros(T + 1, np.int64)
    np.add.at(delta, row_starts, 1)
    np.add.at(delta, row_starts + L, -1)
    return ~(np.cumsum(delta)[:T] > 0)


def run_device(starts: np.ndarray, trace: bool = False):
    """Run the SPMD bass kernel. Returns (mask_u8 [B,T], flags [B], results)."""
    from concourse.bass_utils import run_bass_kernel_spmd

    nc = _get_program()
    shards = starts.reshape(NCORES, RPC, NSEG)
    in_maps = [{"starts": np.ascontiguousarray(shards[c])} for c in range(NCORES)]
    res = run_bass_kernel_spmd(nc, in_maps, list(range(NCORES)), trace=trace)
    mask = np.concatenate([r["mask"] for r in res.results], axis=0)
    occ32 = np.concatenate([r["flags"] for r in res.results], axis=0)  # [B, 32]
    occ = np.bitwise_or.reduce(occ32.astype(np.int64), axis=1)
    flags = ((occ | (-1 << MIN_CLAST)) != -1).astype(np.int32)
    return mask, flags, res


def kernel(**inputs) -> np.ndarray:
    starts = np.ascontiguousarray(np.asarray(inputs["starts"]), dtype=np.int32)
    t_in = int(np.asarray(inputs["T"]))
    l_in = int(np.asarray(inputs["l"]))
    assert starts.shape == (B, NSEG), starts.shape
    assert t_in == T and l_in == L, (t_in, l_in)

    mask_u8, flags, _ = run_device(starts)
    mask = mask_u8.astype(bool)

    bad_rows = np.nonzero(flags != 0)[0]
    for r in bad_rows:  # pathological rows: exact host recompute (rare)
        mask[r] = _host_exact_row(starts[r])
    return mask


# revision 21
# speedup vs baseline: 2.3674x; 1.4334x over previous
"""ContinuousMask kernel for Trainium2 (8 NeuronCores, SPMD row-sharded).

Problem: starts[B=2048, N=8192] int32, T=16384, l=1638. Output bool [B, T]:
True everywhere except the union of windows [s, s+l) over each row's starts.

Algorithm (per row):
  A position t is covered iff some start lies in (t-l, t]. With value-chunks
  of width W=512 (2W <= l), if every chunk 0..28 contains at least one start,
  then the covered region is EXACTLY [smin, smax+l), smin < 512, and
  smax+l >= 15974, so the mask is fully described by a head strip [0, 512)
  (True iff t < smin) and a tail strip [TSTART, T) (True iff t >= smax+l);
  the constant-False middle is never stored (run_bass_kernel_spmd's PJRT
  path donates zero-initialized output buffers). Chunk occupancy is checked
  on a WITNESS SUBSET of columns (passing PROVES the condition; failing only
  flags the row for exact host recompute — on the target distribution a
  512-column witness flags a row with P ~ 5e-7).

Engine split (per 128-row tile), tuned against the TRN2 cost model:
  - loads: 4 quarter-loads, interleaved across both HWDGE queues (SP + Act).
  - DVE (the critical engine, ~1 int32 elem/cycle/lane): min and max via
    fused tensor_tensor_reduce over quarter PAIRS (2 elems/cycle), chained
    through the accumulator-init operand so compute starts when the first
    half lands; witness shift; fused shift-left + or-reduce for occupancy;
    flag test; head-strip paint.
  - Pool (gpsimd): tail-strip paint (integer max/shift are DVE-only).
  - Act (scalar): per-row scalar prep + store DMAs + half the loads.
"""

import numpy as np

B = 2048
T = 16384
NSEG = 8192
L = 1638
NCORES = 8
RPC = B // NCORES  # 256 rows per core
PT = 128  # rows per partition tile
NRT = RPC // PT  # 2 row tiles per core
Q = NSEG // 4  # quarter width (2048)
SHIFT = 9  # occupancy chunk width 512 (2*512 <= L)
OCC_COLS = 512  # occupancy witness column count
# Require witness occupancy of ALL chunks 0..28 (values span [0, 14747), so
# chunk 28 is the last). Chunk 0 occupied => smin < 512; chunk 28 occupied =>
# smax >= 14336 => the tail True-run starts at smax+L >= 15974.
MIN_CLAST = 29
HSTRIP = 512  # head strip [0, 512) covers [0, smin) since smin < 512
TSTART = T - 512  # tail strip [15872, T) covers runs starting >= 15974

_prog_cache: dict = {}


def _register_minmax_ops():
    """Register two-stream min/max reduce custom DVE ops (per-NEFF uop table;
    the documented extension path — no firmware change). The stock
    InstTensorTensorReduce wedges this hardware, so these replace it:
    one instruction streams two operand ranges (2 elems/cycle/lane) and
    folds the pairwise result into the fp32 accumulator."""
    import concourse.dve_ops as dve_ops
    from concourse.dve_ops import DveOp
    from concourse.dve_spec import Spec, Src0, Src1, C0, minn, maxx, lower
    from concourse.dve_spec import _has_src1
    from concourse.dve_uop import DveOpSpec

    existing = {op.name: op for op in dve_ops.OPS}
    if "TT_MIN_REDUCE_X" in existing:
        return existing["TT_MIN_REDUCE_X"], existing["TT_MAX_REDUCE_X"]

    def _ref(np_op):
        fold = np.min if np_op is np.minimum else np.max

        def ref(in0, in1, c0, c1, c2):
            out = np_op(
                np.asarray(in0).astype(np.float32), np.asarray(in1).astype(np.float32)
            )
            acc = np_op(np.asarray(c0, np.float32), fold(out, axis=-1, keepdims=True))
            return out, acc

        return ref

    def make(name, body, accum, np_op):
        spec = Spec(body=body, accum=accum, accum_init=C0, reference=_ref(np_op))
        row = 1 + len(dve_ops.OPS)
        assert row < 0x20, "custom DVE row overflow"
        dve_ops._SUB_OPCODE_FOR_NAME[name] = row
        uops = lower(spec, ver="v3")
        sha = DveOpSpec(
            name=name, opcode=row, uops=uops, rd1_en=_has_src1(spec)
        ).sha("v3")
        op = DveOp(name=name, spec=spec, subdim=False, uops_sha={"v3": sha})
        dve_ops.OPS.append(op)
        dve_ops.CUSTOM_DVE_SPECS[name] = spec
        return op

    mn = make("TT_MIN_REDUCE_X", minn(Src0, Src1), minn, np.minimum)
    mx = make("TT_MAX_REDUCE_X", maxx(Src0, Src1), maxx, np.maximum)
    return mn, mx


def _build_program(reps: int = 1, mode: str = "full"):
    """mode: 'full' | 'dma' (loads+stores only) | 'compute' (load once, compute reps x)."""
    import concourse.bacc as bacc
    import concourse.mybir as mybir
    from concourse.tile import TileContext

    MN_OP, MX_OP = _register_minmax_ops()

    dt = mybir.dt
    Alu = mybir.AluOpType
    X = mybir.AxisListType.X

    nc = bacc.Bacc("TRN2", debug=False)
    starts_d = nc.declare_dram_parameter("starts", [RPC, NSEG], dt.int32, isOutput=False)
    mask_d = nc.declare_dram_parameter("mask", [RPC, T], dt.uint8, isOutput=True)
    flags_d = nc.declare_dram_parameter("flags", [RPC, 32], dt.int32, isOutput=True)

    with TileContext(nc) as tc:
        with (
            tc.tile_pool(name="persist", bufs=1) as pp,
            tc.tile_pool(name="stp", bufs=2) as stp,
            tc.tile_pool(name="scratch", bufs=2) as scp,
            tc.tile_pool(name="strip", bufs=4) as outp,
            tc.tile_pool(name="small", bufs=4) as sp,
        ):
            iota_t = pp.tile([PT, HSTRIP], dt.int16, tag="iota")
            nc.gpsimd.iota(iota_t[:], [[1, HSTRIP]], base=0, channel_multiplier=0)
            ones_t = pp.tile([PT, OCC_COLS], dt.int32, tag="ones")
            nc.vector.memset(ones_t[:], 1)

            persist_st: dict = {}
            for rep in range(reps):
              for rt in range(NRT):
                r0 = rt * PT
                is_compute = mode.startswith("compute")
                parts = mode.split(":")[1] if ":" in mode else "all"
                do_load = not is_compute or rep == 0
                do_compute = mode != "dma"
                do_store = not is_compute

                if is_compute:
                    if rt not in persist_st:
                        st_persist = pp.tile([PT, NSEG], dt.int32, tag=f"st{rt}")
                        persist_st[rt] = st_persist
                    st = persist_st[rt]
                else:
                    st = stp.tile([PT, NSEG], dt.int32, tag="st")
                if do_load:
                    # quarter-loads, interleaved across both HWDGE queues so
                    # the first ttr pair can start at the half-way point
                    nc.sync.dma_start(out=st[:, 0:Q], in_=starts_d[r0 : r0 + PT, 0:Q])
                    nc.scalar.dma_start(out=st[:, Q : 2 * Q], in_=starts_d[r0 : r0 + PT, Q : 2 * Q])
                    nc.sync.dma_start(out=st[:, 2 * Q : 3 * Q], in_=starts_d[r0 : r0 + PT, 2 * Q : 3 * Q])
                    nc.scalar.dma_start(out=st[:, 3 * Q : NSEG], in_=starts_d[r0 : r0 + PT, 3 * Q : NSEG])
                if not do_compute:
                    if do_store:
                        ph0 = outp.tile([PT, HSTRIP], dt.uint8, tag="ph")
                        nc.vector.memset(ph0[:], 0)
                        nc.scalar.dma_start(out=mask_d[r0 : r0 + PT, 0:HSTRIP], in_=ph0[:])
                        pt0 = outp.tile([PT, T - TSTART], dt.uint8, tag="pt")
                        nc.vector.memset(pt0[:], 0)
                        nc.scalar.dma_start(out=mask_d[r0 : r0 + PT, TSTART:T], in_=pt0[:])
                    continue

                # fp32 accumulators: the DVE reduce accumulator is fp32; all
                # values here are < 2^20 so fp32 is exact
                smin = sp.tile([PT, 1], dt.float32, tag="smin")
                smax = sp.tile([PT, 1], dt.float32, tag="smax")
                mn0 = sp.tile([PT, 1], dt.float32, tag="mn0")
                mx0 = sp.tile([PT, 1], dt.float32, tag="mx0")
                dmy = sp.tile([PT, 1], dt.float32, tag="dmy")

                if parts in ("all", "mm"):
                    # min/max over quarter pairs (q0,q1) then (q2,q3), chaining
                    # through the accumulator initial value; the elementwise
                    # output is discarded via a stride-0 broadcast dummy
                    nc.vector._custom_dve(
                        MN_OP, out=dmy.broadcast_to((PT, Q)),
                        in0=st[:, 0:Q], in1=st[:, Q : 2 * Q],
                        s0=float(1 << 20), accum_out=mn0[:],
                    )
                    nc.vector._custom_dve(
                        MX_OP, out=dmy.broadcast_to((PT, Q)),
                        in0=st[:, 0:Q], in1=st[:, Q : 2 * Q],
                        s0=0.0, accum_out=mx0[:],
                    )
                    nc.vector._custom_dve(
                        MN_OP, out=dmy.broadcast_to((PT, Q)),
                        in0=st[:, 2 * Q : 3 * Q], in1=st[:, 3 * Q : NSEG],
                        s0=mn0[:], accum_out=smin[:],
                    )
                    nc.vector._custom_dve(
                        MX_OP, out=dmy.broadcast_to((PT, Q)),
                        in0=st[:, 2 * Q : 3 * Q], in1=st[:, 3 * Q : NSEG],
                        s0=mx0[:], accum_out=smax[:],
                    )
                else:
                    nc.vector.memset(smin[:], 3.0)
                    nc.vector.memset(smax[:], 14500.0)

                if parts in ("all", "occ"):
                    # witness occupancy: shift (needs only q0), bit set, or-tree
                    # down to 32 columns; final OR + flag test on host
                    hi = scp.tile([PT, OCC_COLS], dt.int32, tag="hi")
                    nc.vector.tensor_scalar(hi[:], st[:, 0:OCC_COLS], SHIFT, None, Alu.arith_shift_right)
                    bits = scp.tile([PT, OCC_COLS], dt.int32, tag="bits")
                    nc.vector.tensor_tensor(bits[:], ones_t[:], hi[:], Alu.logical_shift_left)
                    w = OCC_COLS
                    while w > 32:
                        h = w // 2
                        nc.vector.tensor_tensor(
                            bits[:, 0:h], bits[:, 0:h], bits[:, h:w], Alu.bitwise_or
                        )
                        w = h
                    if do_store:
                        nc.scalar.dma_start(out=flags_d[r0 : r0 + PT, :], in_=bits[:, 0:32])

                if parts in ("all", "paint"):
                    # paint strips, all on DVE: head (t < smin), tail
                    # (t >= smax + L - TSTART); the tail threshold is a tiny
                    # per-partition add
                    smaxl_f = sp.tile([PT, 1], dt.float32, tag="smaxlf")
                    nc.vector.tensor_scalar(
                        smaxl_f[:], smax[:], float(L - TSTART), None, Alu.add
                    )
                    ph = outp.tile([PT, HSTRIP], dt.uint8, tag="ph")
                    pt = outp.tile([PT, T - TSTART], dt.uint8, tag="pt")
                    nc.vector.tensor_scalar(ph[:], iota_t[:], smin[:], None, Alu.is_lt)
                    nc.vector.tensor_scalar(pt[:], iota_t[:], smaxl_f[:], None, Alu.is_ge)
                    if do_store:
                        nc.scalar.dma_start(out=mask_d[r0 : r0 + PT, 0:HSTRIP], in_=ph[:])
                        nc.scalar.dma_start(out=mask_d[r0 : r0 + PT, TSTART:T], in_=pt[:])

    nc.finalize()
    return nc


def _get_program(reps: int = 1, mode: str = "full"):
    key = (reps, mode)
    if key not in _prog_cache:
        _prog_cache[key] = _build_program(reps, mode)
    return _prog_cache[key]


def _host_exact_row(row_starts: np.ndarray) -> np.ndarray:
    delta = np.zeros(T + 1, np.int64)
    np.add.at(delta, row_starts, 1)
    np.add.at(delta, row_starts + L, -1)
    return ~(np.cumsum(delta)[:T] > 0)


def run_device(starts: np.ndarray, trace: bool = False):
    """Run the SPMD bass kernel. Returns (mask_u8 [B,T], flags [B], results)."""
    from concourse.bass_utils import run_bass_kernel_spmd

    nc = _get_program()
    shards = starts.reshape(NCORES, RPC, NSEG)
    in_maps = [{"starts": np.ascontiguousarray(shards[c])} for c in range(NCORES)]
    res = run_bass_kernel_spmd(nc, in_maps, list(range(NCORES)), trace=trace)
    mask = np.concatenate([r["mask"] for r in res.results], axis=0)
    occ32 = np.concatenate([r["flags"] for r in res.results], axis=0)  # [B, 32]
    occ = np.bitwise_or.reduce(occ32.astype(np.int64), axis=1)
    flags = ((occ | (-1 << MIN_CLAST)) != -1).astype(np.int32)
    return mask, flags, res


def kernel(**inputs) -> np.ndarray:
    starts = np.ascontiguousarray(np.asarray(inputs["starts"]), dtype=np.int32)
    t_in = int(np.asarray(inputs["T"]))
    l_in = int(np.asarray(inputs["l"]))
    assert starts.shape == (B, NSEG), starts.shape
    assert t_in == T and l_in == L, (t_in, l_in)

    mask_u8, flags, _ = run_device(starts)
    mask = mask_u8.astype(bool)

    bad_rows = np.nonzero(flags != 0)[0]
    for r in bad_rows:  # pathological rows: exact host recompute (rare)
        mask[r] = _host_exact_row(starts[r])
    return mask
